# revision 2
# baseline (speedup 1.0000x reference)
"""Cost-volume layer (17-shift cross pattern, R=4) for Trainium2, 8 NeuronCores.

out[b,s,h,w] = sum_c src[b,c,h,w] * tgt[b,c,h+dh_s,w+dw_s]   (tgt zero-padded)

Strategy
--------
Shard: 8 cores = batch(4) x H-halves(2). Per core: src [128, 48*160],
tgt (padded, with halo) [128, 56*168]. C=128 lives in the SBUF partition
dim and is contracted on the TensorEngine via *banded correlations*:

- vertical shifts  (dh=-4..4, dw=0): per column w, matmul
    src[:, :, w]^T @ tgt[:, :, w+4]  ->  [48, 56] band matrix
- horizontal shifts (dh=0, dw=-4..4): per row h, per 32-col chunk i, matmul
    src[:, h, 32i:32i+32]^T @ tgt[:, h+4, 32i:32i+40] -> [32, 40] band matrix

Useful entries are the 9 diagonals of each band; bands are packed into PSUM
banks (several chunks per bank at 32-aligned partition bases), staged to SBUF
(DVE/ACT copies), DMA'd to HBM, and the diagonals are gathered host-side
(pure indexing - no host arithmetic).
"""

import numpy as np
from contextlib import ExitStack

import concourse.bacc as bacc
import concourse.tile as tile
from concourse import mybir
from concourse import bass_utils

R = 4
B, C, H, W = 4, 128, 96, 160
NCORES = 8
HSH = H // 2            # 48 output rows per shard
HT = HSH + 2 * R        # 56 tgt rows (with halo)
WP = W + 2 * R          # 168 padded width
F32 = mybir.dt.float32

# vertical pass: per-w matmul M=48, N=56; pack 2 groups (part base 0, 64)
# x 9 w-slots per PSUM bank -> 18 w per bank
VSLOT = 9
VBASES = (0, 64)
VPERBANK = VSLOT * len(VBASES)          # 18
NVBANK = (W + VPERBANK - 1) // VPERBANK  # 9

# horizontal pass: chunks of 32 src cols, window N=40; pack 4 groups
# (bases 0,32,64,96) x 12 slots per bank -> 48 chunks per bank
MH = 32
NH = MH + 2 * R         # 40
NCH = W // MH           # 5 chunks per row
NQ = HSH * NCH          # 240 chunks total
HSLOT = 12
HBASES = (0, 32, 64, 96)
HPERBANK = HSLOT * len(HBASES)          # 48
NHBANK = (NQ + HPERBANK - 1) // HPERBANK  # 5

SHIFTS = [(0, 0)]
for i in range(1, R + 1):
    SHIFTS.extend([(-i, 0), (i, 0), (0, -i), (0, i)])


def build_nc():
    nc = bacc.Bacc("TRN2", target_bir_lowering=False)
    src = nc.dram_tensor("src", [C, HSH * W], F32, kind="ExternalInput")
    tgt = nc.dram_tensor("tgt", [C, HT * WP], F32, kind="ExternalInput")
    vband = nc.dram_tensor("vband", [HSH, W, HT], F32, kind="ExternalOutput")
    hband = nc.dram_tensor("hband", [MH, NQ, NH], F32, kind="ExternalOutput")

    with ExitStack() as ctx:
        tc = ctx.enter_context(tile.TileContext(nc))
        ins = ctx.enter_context(tc.tile_pool(name="ins", bufs=1))
        psum = ctx.enter_context(tc.tile_pool(name="psum", bufs=4, space="PSUM"))
        stage = ctx.enter_context(tc.tile_pool(name="stage", bufs=4))

        src_sb = ins.tile([C, HSH * W], F32)
        tgt_sb = ins.tile([C, HT * WP], F32)

        # load inputs in two pieces each so early compute can overlap tail DMA
        t_cut = 32 * WP   # tgt padded rows 0..31 (covers real rows -4..27)
        s_cut = 24 * W    # src rows 0..23
        nc.sync.dma_start(out=tgt_sb[:, :t_cut], in_=tgt[:][:, :t_cut])
        nc.sync.dma_start(out=src_sb[:, :s_cut], in_=src[:][:, :s_cut])
        nc.sync.dma_start(out=tgt_sb[:, t_cut:], in_=tgt[:][:, t_cut:])
        nc.sync.dma_start(out=src_sb[:, s_cut:], in_=src[:][:, s_cut:])

        src3 = src_sb.rearrange("c (h w) -> c h w", w=W)
        tgt3 = tgt_sb.rearrange("c (h w) -> c h w", w=WP)

        copy_flip = [0]

        def stage_copy(dst, src_ap):
            # alternate PSUM->SBUF copies between DVE and ACT
            if copy_flip[0] % 2 == 0:
                nc.vector.tensor_copy(out=dst, in_=src_ap)
            else:
                nc.scalar.copy(out=dst, in_=src_ap)
            copy_flip[0] += 1

        def horiz_bank(bank):
            q0 = bank * HPERBANK
            pt = psum.tile([128, HSLOT * NH], F32, tag="hp")
            for g, base in enumerate(HBASES):
                for j in range(HSLOT):
                    q = q0 + g * HSLOT + j
                    h, i = divmod(q, NCH)
                    w0 = i * MH
                    nc.tensor.matmul(
                        out=pt[base:base + MH, j * NH:(j + 1) * NH],
                        lhsT=src3[:, h, w0:w0 + MH],
                        rhs=tgt3[:, h + R, w0:w0 + NH],
                        start=True, stop=True,
                        tile_position=(0, base),
                    )
            st = stage.tile([128, HSLOT * NH], F32, tag="hs")
            stage_copy(st, pt)
            for g in range(len(HBASES)):
                qa = q0 + g * HSLOT
                nc.sync.dma_start(
                    out=hband[:][:, qa:qa + HSLOT, :],
                    in_=st[HBASES[g]:HBASES[g] + MH, :],
                )

        def vert_bank(bank):
            w0 = bank * VPERBANK
            nw = min(VPERBANK, W - w0)
            pt = psum.tile([112, VSLOT * HT], F32, tag="vp")
            for g, base in enumerate(VBASES):
                ng = min(VSLOT, max(0, nw - g * VSLOT))
                for j in range(ng):
                    w = w0 + g * VSLOT + j
                    nc.tensor.matmul(
                        out=pt[base:base + HSH, j * HT:(j + 1) * HT],
                        lhsT=src3[:, :, w],
                        rhs=tgt3[:, 0:HT, w + R],
                        start=True, stop=True,
                        tile_position=(0, base),
                    )
            st = stage.tile([112, VSLOT * HT], F32, tag="vs")
            for g, base in enumerate(VBASES):
                ng = min(VSLOT, max(0, nw - g * VSLOT))
                if ng > 0:
                    stage_copy(
                        st[base:base + HSH, :ng * HT],
                        pt[base:base + HSH, :ng * HT],
                    )
            for g, base in enumerate(VBASES):
                ng = min(VSLOT, max(0, nw - g * VSLOT))
                if ng > 0:
                    wa = w0 + g * VSLOT
                    nc.sync.dma_start(
                        out=vband[:][:, wa:wa + ng, :],
                        in_=st[base:base + HSH, :ng * HT],
                    )

        # horizontal banks 0-1 only need src rows <= 19 / tgt rows <= 23+4:
        # they can start while the tail input DMAs are still in flight
        for bank in range(2):
            horiz_bank(bank)
        for bank in range(NVBANK):
            vert_bank(bank)
        for bank in range(2, NHBANK):
            horiz_bank(bank)

    nc.compile()
    return nc


_NC_CACHE = []


def _get_nc():
    if not _NC_CACHE:
        _NC_CACHE.append(build_nc())
    return _NC_CACHE[0]


def shard_inputs(src, tgt):
    src = np.asarray(src, dtype=np.float32)
    tgt = np.asarray(tgt, dtype=np.float32)
    tp = np.pad(tgt, ((0, 0), (0, 0), (R, R), (R, R)))
    in_maps = []
    for core in range(NCORES):
        b, hh = divmod(core, 2)
        h0 = hh * HSH
        s = np.ascontiguousarray(src[b, :, h0:h0 + HSH, :]).reshape(C, HSH * W)
        t = np.ascontiguousarray(tp[b, :, h0:h0 + HT, :]).reshape(C, HT * WP)
        in_maps.append({"src": s, "tgt": t})
    return in_maps


def extract_output(results):
    """results: list of 8 dicts with 'vband' [48,160,56], 'hband' [32,240,40]."""
    out = np.zeros((B, len(SHIFTS), H, W), np.float32)
    hidx = np.arange(HSH)
    midx = np.arange(MH)
    widx = np.arange(W)
    iidx = np.arange(NCH)
    for core in range(NCORES):
        b, hh = divmod(core, 2)
        h0 = hh * HSH
        vb = np.asarray(results[core]["vband"])
        hb = np.asarray(results[core]["hband"]).reshape(MH, HSH, NCH, NH)
        for s, (dh, dw) in enumerate(SHIFTS):
            if dw == 0:
                out[b, s, h0:h0 + HSH, :] = vb[
                    hidx[:, None], widx[None, :], (hidx + dh + R)[:, None]
                ]
            else:
                v = hb[
                    midx[:, None, None],
                    hidx[None, :, None],
                    iidx[None, None, :],
                    (midx + dw + R)[:, None, None],
                ]  # [m, h, i]
                out[b, s, h0:h0 + HSH, :] = v.transpose(1, 2, 0).reshape(HSH, W)
    return out


def kernel(src, tgt, **run_kwargs):
    nc = _get_nc()
    in_maps = shard_inputs(src, tgt)
    res = bass_utils.run_bass_kernel_spmd(
        nc, in_maps, core_ids=list(range(NCORES)), **run_kwargs
    )
    out = extract_output(res.results)
    kernel.last_result = res
    return out


# revision 3
# speedup vs baseline: 1.0051x; 1.0051x over previous
"""Cost-volume layer (17-shift cross pattern, R=4) for Trainium2, 8 NeuronCores.

out[b,s,h,w] = sum_c src[b,c,h,w] * tgt[b,c,h+dh_s,w+dw_s]   (tgt zero-padded)

Strategy
--------
Shard: 8 cores = batch(4) x H-halves(2). Per core: src [128, 48*160],
tgt (padded, with halo) [128, 56*168]. C=128 lives in the SBUF partition
dim and is contracted on the TensorEngine via *banded correlations*:

- vertical shifts  (dh=-4..4, dw=0): per column w, matmul
    src[:, :, w]^T @ tgt[:, :, w+4]  ->  [48, 56] band matrix
- horizontal shifts (dh=0, dw=-4..4): per row h, per 32-col chunk i, matmul
    src[:, h, 32i:32i+32]^T @ tgt[:, h+4, 32i:32i+40] -> [32, 40] band matrix

Useful entries are the 9 diagonals of each band; bands are packed into PSUM
banks (several chunks per bank at 32-aligned partition bases), staged to SBUF
(DVE/ACT copies), DMA'd to HBM, and the diagonals are gathered host-side
(pure indexing - no host arithmetic).

Input loads are sliced by row-groups so horizontal banks start while the
tail of the inputs is still loading; band writes go out on the ACT HWDGE
ring so they do not serialize against input loads on the sync ring.
"""

import numpy as np
from contextlib import ExitStack

import concourse.bacc as bacc
import concourse.tile as tile
from concourse import mybir
from concourse import bass_utils

R = 4
B, C, H, W = 4, 128, 96, 160
NCORES = 8
HSH = H // 2            # 48 output rows per shard
HT = HSH + 2 * R        # 56 tgt rows (with halo)
WP = W + 2 * R          # 168 padded width
F32 = mybir.dt.float32

# compute dtype for the matmul inputs ("float32" safe, "bfloat16" fast)
COMPUTE_DT = "float32"

# vertical pass: per-w matmul M=48, N=56; pack 2 groups (part base 0, 64)
# x 9 w-slots per PSUM bank -> 18 w per bank
VSLOT = 9
VBASES = (0, 64)
VPERBANK = VSLOT * len(VBASES)          # 18
NVBANK = (W + VPERBANK - 1) // VPERBANK  # 9

# horizontal pass: chunks of 32 src cols, window N=40; pack 4 groups
# (bases 0,32,64,96) x 12 slots per bank -> 48 chunks per bank
MH = 32
NH = MH + 2 * R         # 40
NCH = W // MH           # 5 chunks per row
NQ = HSH * NCH          # 240 chunks total
HSLOT = 12
HBASES = (0, 32, 64, 96)
HPERBANK = HSLOT * len(HBASES)          # 48
NHBANK = (NQ + HPERBANK - 1) // HPERBANK  # 5

SHIFTS = [(0, 0)]
for i in range(1, R + 1):
    SHIFTS.extend([(-i, 0), (i, 0), (0, -i), (0, i)])

# input load row-slices: tgt rows [0,16,32,44,56), src rows [0,12,24,36,48)
TGT_CUTS = [0, 16, 32, 44, 56]
SRC_CUTS = [0, 12, 24, 36, 48]
# horizontal bank b covers h in [b*48/5, ...]; bank ready after these pieces:
#   bank0: h<=9  -> tgt rows <=13 (piece 1), src rows <=9  (piece 1)
#   bank1: h<=19 -> tgt <=23 (piece 2), src <=19 (piece 2)
#   bank2: h<=28 -> tgt <=32 (piece 3), src <=28 (piece 3)
#   bank3: h<=38 -> tgt <=42 (piece 3), src <=38 (piece 4)
#   bank4: h<=47 -> all


def build_nc():
    cdt = getattr(mybir.dt, COMPUTE_DT)
    nc = bacc.Bacc("TRN2", target_bir_lowering=False)
    src = nc.dram_tensor("src", [C, HSH * W], cdt, kind="ExternalInput")
    tgt = nc.dram_tensor("tgt", [C, HT * WP], cdt, kind="ExternalInput")
    vband = nc.dram_tensor("vband", [HSH, W, HT], F32, kind="ExternalOutput")
    hband = nc.dram_tensor("hband", [MH, NQ, NH], F32, kind="ExternalOutput")

    with ExitStack() as ctx:
        tc = ctx.enter_context(tile.TileContext(nc))
        ins = ctx.enter_context(tc.tile_pool(name="ins", bufs=1))
        psum = ctx.enter_context(tc.tile_pool(name="psum", bufs=4, space="PSUM"))
        stage = ctx.enter_context(tc.tile_pool(name="stage", bufs=6))

        src_sb = ins.tile([C, HSH * W], cdt)
        tgt_sb = ins.tile([C, HT * WP], cdt)

        def load_piece(i):
            t0, t1 = TGT_CUTS[i] * WP, TGT_CUTS[i + 1] * WP
            s0, s1 = SRC_CUTS[i] * W, SRC_CUTS[i + 1] * W
            nc.sync.dma_start(out=tgt_sb[:, t0:t1], in_=tgt[:][:, t0:t1])
            nc.sync.dma_start(out=src_sb[:, s0:s1], in_=src[:][:, s0:s1])

        src3 = src_sb.rearrange("c (h w) -> c h w", w=W)
        tgt3 = tgt_sb.rearrange("c (h w) -> c h w", w=WP)

        copy_flip = [0]

        def stage_copy(dst, src_ap):
            # alternate PSUM->SBUF copies between DVE and ACT
            if copy_flip[0] % 2 == 0:
                nc.vector.tensor_copy(out=dst, in_=src_ap)
            else:
                nc.scalar.copy(out=dst, in_=src_ap)
            copy_flip[0] += 1

        def horiz_bank(bank):
            q0 = bank * HPERBANK
            pt = psum.tile([128, HSLOT * NH], F32, tag="hp")
            for g, base in enumerate(HBASES):
                for j in range(HSLOT):
                    q = q0 + g * HSLOT + j
                    h, i = divmod(q, NCH)
                    w0 = i * MH
                    nc.tensor.matmul(
                        out=pt[base:base + MH, j * NH:(j + 1) * NH],
                        lhsT=src3[:, h, w0:w0 + MH],
                        rhs=tgt3[:, h + R, w0:w0 + NH],
                        start=True, stop=True,
                        tile_position=(0, base),
                    )
            st = stage.tile([128, HSLOT * NH], F32, tag="hs")
            stage_copy(st, pt)
            for g in range(len(HBASES)):
                qa = q0 + g * HSLOT
                nc.scalar.dma_start(
                    out=hband[:][:, qa:qa + HSLOT, :],
                    in_=st[HBASES[g]:HBASES[g] + MH, :],
                )

        def vert_bank(bank):
            w0 = bank * VPERBANK
            nw = min(VPERBANK, W - w0)
            pt = psum.tile([112, VSLOT * HT], F32, tag="vp")
            for g, base in enumerate(VBASES):
                ng = min(VSLOT, max(0, nw - g * VSLOT))
                for j in range(ng):
                    w = w0 + g * VSLOT + j
                    nc.tensor.matmul(
                        out=pt[base:base + HSH, j * HT:(j + 1) * HT],
                        lhsT=src3[:, :, w],
                        rhs=tgt3[:, 0:HT, w + R],
                        start=True, stop=True,
                        tile_position=(0, base),
                    )
            st = stage.tile([112, VSLOT * HT], F32, tag="vs")
            for g, base in enumerate(VBASES):
                ng = min(VSLOT, max(0, nw - g * VSLOT))
                if ng > 0:
                    stage_copy(
                        st[base:base + HSH, :ng * HT],
                        pt[base:base + HSH, :ng * HT],
                    )
            for g, base in enumerate(VBASES):
                ng = min(VSLOT, max(0, nw - g * VSLOT))
                if ng > 0:
                    wa = w0 + g * VSLOT
                    nc.scalar.dma_start(
                        out=vband[:][:, wa:wa + ng, :],
                        in_=st[base:base + HSH, :ng * HT],
                    )

        # pipeline: issue loads piecewise; horizontal banks unlock as the
        # rows they need land; vertical banks need everything.
        load_piece(0)
        load_piece(1)
        horiz_bank(0)
        load_piece(2)
        horiz_bank(1)
        load_piece(3)
        horiz_bank(2)
        horiz_bank(3)
        horiz_bank(4)
        for bank in range(NVBANK):
            vert_bank(bank)

    nc.compile()
    return nc


_NC_CACHE = []


def _get_nc():
    if not _NC_CACHE:
        _NC_CACHE.append(build_nc())
    return _NC_CACHE[0]


def shard_inputs(src, tgt):
    np_cdt = np.float32 if COMPUTE_DT == "float32" else np.dtype("bfloat16")
    src = np.asarray(src, dtype=np.float32)
    tgt = np.asarray(tgt, dtype=np.float32)
    tp = np.pad(tgt, ((0, 0), (0, 0), (R, R), (R, R)))
    in_maps = []
    for core in range(NCORES):
        b, hh = divmod(core, 2)
        h0 = hh * HSH
        s = np.ascontiguousarray(src[b, :, h0:h0 + HSH, :]).reshape(C, HSH * W)
        t = np.ascontiguousarray(tp[b, :, h0:h0 + HT, :]).reshape(C, HT * WP)
        in_maps.append({"src": s.astype(np_cdt), "tgt": t.astype(np_cdt)})
    return in_maps


def extract_output(results):
    """results: list of 8 dicts with 'vband' [48,160,56], 'hband' [32,240,40]."""
    out = np.zeros((B, len(SHIFTS), H, W), np.float32)
    hidx = np.arange(HSH)
    midx = np.arange(MH)
    widx = np.arange(W)
    iidx = np.arange(NCH)
    for core in range(NCORES):
        b, hh = divmod(core, 2)
        h0 = hh * HSH
        vb = np.asarray(results[core]["vband"])
        hb = np.asarray(results[core]["hband"]).reshape(MH, HSH, NCH, NH)
        for s, (dh, dw) in enumerate(SHIFTS):
            if dw == 0:
                out[b, s, h0:h0 + HSH, :] = vb[
                    hidx[:, None], widx[None, :], (hidx + dh + R)[:, None]
                ]
            else:
                v = hb[
                    midx[:, None, None],
                    hidx[None, :, None],
                    iidx[None, None, :],
                    (midx + dw + R)[:, None, None],
                ]  # [m, h, i]
                out[b, s, h0:h0 + HSH, :] = v.transpose(1, 2, 0).reshape(HSH, W)
    return out


def kernel(src, tgt, **run_kwargs):
    nc = _get_nc()
    in_maps = shard_inputs(src, tgt)
    res = bass_utils.run_bass_kernel_spmd(
        nc, in_maps, core_ids=list(range(NCORES)), **run_kwargs
    )
    out = extract_output(res.results)
    kernel.last_result = res
    return out


# revision 5
# speedup vs baseline: 1.4618x; 1.4543x over previous
"""Cost-volume layer (17-shift cross pattern, R=4) for Trainium2, 8 NeuronCores.

out[b,s,h,w] = sum_c src[b,c,h,w] * tgt[b,c,h+dh_s,w+dw_s]   (tgt zero-padded)

Strategy
--------
Shard: 8 cores = batch(4) x H-halves(2). Per core: src [128, 48*160],
tgt (padded, with halo) [128, 56*168]. C=128 lives in the SBUF partition
dim and is contracted on the TensorEngine via *banded correlations*:

- vertical shifts  (dh=-4..4, dw=0): per column w, matmul
    src[:, :, w]^T @ tgt[:, :, w+4]  ->  [48, 56] band matrix
- horizontal shifts (dh=0, dw=-4..4): per row h, per 32-col chunk i, matmul
    src[:, h, 32i:32i+32]^T @ tgt[:, h+4, 32i:32i+40] -> [32, 40] band matrix

Useful entries are the 9 diagonals of each band; bands are packed into PSUM
banks (several chunks per bank at 32-aligned partition bases), staged to SBUF
(DVE/ACT copies), DMA'd to HBM, and the diagonals are gathered host-side
(pure indexing - no host arithmetic).

Input loads are sliced by row-groups so horizontal banks start while the
tail of the inputs is still loading; band writes go out on the ACT HWDGE
ring so they do not serialize against input loads on the sync ring.
"""

import numpy as np
from contextlib import ExitStack

import concourse.bacc as bacc
import concourse.tile as tile
from concourse import mybir
from concourse import bass_utils

R = 4
B, C, H, W = 4, 128, 96, 160
NCORES = 8
HSH = H // 2            # 48 output rows per shard
HT = HSH + 2 * R        # 56 tgt rows (with halo)
WP = W + 2 * R          # 168 padded width
F32 = mybir.dt.float32

# compute dtype for the matmul inputs. float16 runs the PE at full rate
# (1 cyc/row vs fp32's 4) and halves input DMA, at ~3e-4 relative error
# (randn inputs are far from fp16 overflow). "float32" is the exact fallback.
COMPUTE_DT = "float16"

# vertical pass: per-w matmul M=48, N=56; pack 2 groups (part base 0, 64)
# x 9 w-slots per PSUM bank -> 18 w per bank
VSLOT = 9
VBASES = (0, 64)
VPERBANK = VSLOT * len(VBASES)          # 18
NVBANK = (W + VPERBANK - 1) // VPERBANK  # 9

# horizontal pass: chunks of 32 src cols, window N=40; pack 4 groups
# (bases 0,32,64,96) x 12 slots per bank -> 48 chunks per bank
MH = 32
NH = MH + 2 * R         # 40
NCH = W // MH           # 5 chunks per row
NQ = HSH * NCH          # 240 chunks total
HSLOT = 12
HBASES = (0, 32, 64, 96)
HPERBANK = HSLOT * len(HBASES)          # 48
NHBANK = (NQ + HPERBANK - 1) // HPERBANK  # 5

SHIFTS = [(0, 0)]
for i in range(1, R + 1):
    SHIFTS.extend([(-i, 0), (i, 0), (0, -i), (0, i)])

# input load row-slices: tgt rows [0,16,32,44,56), src rows [0,12,24,36,48)
TGT_CUTS = [0, 16, 32, 44, 56]
SRC_CUTS = [0, 12, 24, 36, 48]
# horizontal bank b covers h in [b*48/5, ...]; bank ready after these pieces:
#   bank0: h<=9  -> tgt rows <=13 (piece 1), src rows <=9  (piece 1)
#   bank1: h<=19 -> tgt <=23 (piece 2), src <=19 (piece 2)
#   bank2: h<=28 -> tgt <=32 (piece 3), src <=28 (piece 3)
#   bank3: h<=38 -> tgt <=42 (piece 3), src <=38 (piece 4)
#   bank4: h<=47 -> all


def build_nc():
    cdt = getattr(mybir.dt, COMPUTE_DT)
    nc = bacc.Bacc("TRN2", target_bir_lowering=False)
    src = nc.dram_tensor("src", [C, HSH * W], cdt, kind="ExternalInput")
    tgt = nc.dram_tensor("tgt", [C, HT * WP], cdt, kind="ExternalInput")
    vband = nc.dram_tensor("vband", [HSH, W, HT], F32, kind="ExternalOutput")
    hband = nc.dram_tensor("hband", [MH, NQ, NH], F32, kind="ExternalOutput")

    with ExitStack() as ctx:
        tc = ctx.enter_context(tile.TileContext(nc))
        ins = ctx.enter_context(tc.tile_pool(name="ins", bufs=1))
        psum = ctx.enter_context(tc.tile_pool(name="psum", bufs=4, space="PSUM"))
        stage = ctx.enter_context(tc.tile_pool(name="stage", bufs=6))

        src_sb = ins.tile([C, HSH * W], cdt)
        tgt_sb = ins.tile([C, HT * WP], cdt)

        def load_piece(i):
            t0, t1 = TGT_CUTS[i] * WP, TGT_CUTS[i + 1] * WP
            s0, s1 = SRC_CUTS[i] * W, SRC_CUTS[i + 1] * W
            nc.sync.dma_start(out=tgt_sb[:, t0:t1], in_=tgt[:][:, t0:t1])
            nc.sync.dma_start(out=src_sb[:, s0:s1], in_=src[:][:, s0:s1])

        src3 = src_sb.rearrange("c (h w) -> c h w", w=W)
        tgt3 = tgt_sb.rearrange("c (h w) -> c h w", w=WP)

        copy_flip = [0]

        def stage_copy(dst, src_ap):
            # alternate PSUM->SBUF copies between DVE and ACT
            if copy_flip[0] % 2 == 0:
                nc.vector.tensor_copy(out=dst, in_=src_ap)
            else:
                nc.scalar.copy(out=dst, in_=src_ap)
            copy_flip[0] += 1

        def horiz_bank(bank):
            q0 = bank * HPERBANK
            pt = psum.tile([128, HSLOT * NH], F32, tag="hp")
            for g, base in enumerate(HBASES):
                for j in range(HSLOT):
                    q = q0 + g * HSLOT + j
                    h, i = divmod(q, NCH)
                    w0 = i * MH
                    nc.tensor.matmul(
                        out=pt[base:base + MH, j * NH:(j + 1) * NH],
                        lhsT=src3[:, h, w0:w0 + MH],
                        rhs=tgt3[:, h + R, w0:w0 + NH],
                        start=True, stop=True,
                        tile_position=(0, base),
                    )
            st = stage.tile([128, HSLOT * NH], F32, tag="hs")
            stage_copy(st, pt)
            for g in range(len(HBASES)):
                qa = q0 + g * HSLOT
                nc.scalar.dma_start(
                    out=hband[:][:, qa:qa + HSLOT, :],
                    in_=st[HBASES[g]:HBASES[g] + MH, :],
                )

        def vert_bank(bank):
            w0 = bank * VPERBANK
            nw = min(VPERBANK, W - w0)
            pt = psum.tile([112, VSLOT * HT], F32, tag="vp")
            for g, base in enumerate(VBASES):
                ng = min(VSLOT, max(0, nw - g * VSLOT))
                for j in range(ng):
                    w = w0 + g * VSLOT + j
                    nc.tensor.matmul(
                        out=pt[base:base + HSH, j * HT:(j + 1) * HT],
                        lhsT=src3[:, :, w],
                        rhs=tgt3[:, 0:HT, w + R],
                        start=True, stop=True,
                        tile_position=(0, base),
                    )
            st = stage.tile([112, VSLOT * HT], F32, tag="vs")
            for g, base in enumerate(VBASES):
                ng = min(VSLOT, max(0, nw - g * VSLOT))
                if ng > 0:
                    stage_copy(
                        st[base:base + HSH, :ng * HT],
                        pt[base:base + HSH, :ng * HT],
                    )
            for g, base in enumerate(VBASES):
                ng = min(VSLOT, max(0, nw - g * VSLOT))
                if ng > 0:
                    wa = w0 + g * VSLOT
                    nc.scalar.dma_start(
                        out=vband[:][:, wa:wa + ng, :],
                        in_=st[base:base + HSH, :ng * HT],
                    )

        # pipeline: issue loads piecewise; horizontal banks unlock as the
        # rows they need land; vertical banks need everything.
        load_piece(0)
        load_piece(1)
        horiz_bank(0)
        load_piece(2)
        horiz_bank(1)
        load_piece(3)
        horiz_bank(2)
        horiz_bank(3)
        horiz_bank(4)
        for bank in range(NVBANK):
            vert_bank(bank)

    nc.compile()
    return nc


_NC_CACHE = []


def _get_nc():
    if not _NC_CACHE:
        _NC_CACHE.append(build_nc())
    return _NC_CACHE[0]


def shard_inputs(src, tgt):
    if COMPUTE_DT == "float32":
        np_cdt = np.float32
    elif COMPUTE_DT == "float16":
        np_cdt = np.float16
    else:
        import ml_dtypes
        np_cdt = np.dtype(ml_dtypes.bfloat16)
    src = np.asarray(src, dtype=np.float32)
    tgt = np.asarray(tgt, dtype=np.float32)
    tp = np.pad(tgt, ((0, 0), (0, 0), (R, R), (R, R)))
    in_maps = []
    for core in range(NCORES):
        b, hh = divmod(core, 2)
        h0 = hh * HSH
        s = np.ascontiguousarray(src[b, :, h0:h0 + HSH, :]).reshape(C, HSH * W)
        t = np.ascontiguousarray(tp[b, :, h0:h0 + HT, :]).reshape(C, HT * WP)
        in_maps.append({"src": s.astype(np_cdt), "tgt": t.astype(np_cdt)})
    return in_maps


def extract_output(results):
    """results: list of 8 dicts with 'vband' [48,160,56], 'hband' [32,240,40]."""
    out = np.zeros((B, len(SHIFTS), H, W), np.float32)
    hidx = np.arange(HSH)
    midx = np.arange(MH)
    widx = np.arange(W)
    iidx = np.arange(NCH)
    for core in range(NCORES):
        b, hh = divmod(core, 2)
        h0 = hh * HSH
        vb = np.asarray(results[core]["vband"])
        hb = np.asarray(results[core]["hband"]).reshape(MH, HSH, NCH, NH)
        for s, (dh, dw) in enumerate(SHIFTS):
            if dw == 0:
                out[b, s, h0:h0 + HSH, :] = vb[
                    hidx[:, None], widx[None, :], (hidx + dh + R)[:, None]
                ]
            else:
                v = hb[
                    midx[:, None, None],
                    hidx[None, :, None],
                    iidx[None, None, :],
                    (midx + dw + R)[:, None, None],
                ]  # [m, h, i]
                out[b, s, h0:h0 + HSH, :] = v.transpose(1, 2, 0).reshape(HSH, W)
    return out


def kernel(src, tgt, **run_kwargs):
    nc = _get_nc()
    in_maps = shard_inputs(src, tgt)
    res = bass_utils.run_bass_kernel_spmd(
        nc, in_maps, core_ids=list(range(NCORES)), **run_kwargs
    )
    out = extract_output(res.results)
    kernel.last_result = res
    return out


# revision 8
# speedup vs baseline: 1.7764x; 1.2152x over previous
"""Cost-volume layer (17-shift cross pattern, R=4) for Trainium2, 8 NeuronCores.

out[b,s,h,w] = sum_c src[b,c,h,w] * tgt[b,c,h+dh_s,w+dw_s]   (tgt zero-padded)

Strategy
--------
Shard: 8 cores = batch(4) x H-halves(2). Per core: src [128, 48*160],
tgt (padded, with halo) [128, 56*168]. C=128 lives in the SBUF partition
dim and is contracted on the TensorEngine via *banded correlations*:

- vertical shifts  (dh=-4..4, dw=0): per column w, matmul
    src[:, :, w]^T @ tgt[:, :, w+4]  ->  [48, 56] band matrix
- horizontal shifts (dh=0, dw=-4..4): per row h, per 32-col chunk i, matmul
    src[:, h, 32i:32i+32]^T @ tgt[:, h+4, 32i:32i+40] -> [32, 40] band matrix

Useful entries are the 9 diagonals of each band; bands are packed into PSUM
banks (several chunks per bank at 32-aligned partition bases), staged to SBUF
(DVE/ACT copies), DMA'd to HBM, and the diagonals are gathered host-side
(pure indexing - no host arithmetic).

Input loads are sliced by row-groups so horizontal banks start while the
tail of the inputs is still loading; band writes go out on the ACT HWDGE
ring so they do not serialize against input loads on the sync ring.
"""

import numpy as np
from contextlib import ExitStack

import concourse.bacc as bacc
import concourse.tile as tile
from concourse import mybir
from concourse import bass_utils

R = 4
B, C, H, W = 4, 128, 96, 160
NCORES = 8
HSH = H // 2            # 48 output rows per shard
HT = HSH + 2 * R        # 56 tgt rows (with halo)
WP = W + 2 * R          # 168 padded width
F32 = mybir.dt.float32

# compute dtype for the matmul inputs. float16 runs the PE at full rate
# (1 cyc/row vs fp32's 4) and halves input DMA, at ~3e-4 relative error
# (randn inputs are far from fp16 overflow). "float32" is the exact fallback.
COMPUTE_DT = "float16"

# vertical pass: per-w matmul M=48, N=56; pack 2 groups (part base 0, 64)
# x 9 w-slots per PSUM bank -> 18 w per bank
VSLOT = 9
VBASES = (0, 64)
VPERBANK = VSLOT * len(VBASES)          # 18
NVBANK = (W + VPERBANK - 1) // VPERBANK  # 9

# horizontal pass: chunks of 32 src cols, window N=40; pack 4 groups
# (bases 0,32,64,96) x 12 slots per bank -> 48 chunks per bank
MH = 32
NH = MH + 2 * R         # 40
NCH = W // MH           # 5 chunks per row
NQ = HSH * NCH          # 240 chunks total
HSLOT = 12
HBASES = (0, 32, 64, 96)
HPERBANK = HSLOT * len(HBASES)          # 48
NHBANK = (NQ + HPERBANK - 1) // HPERBANK  # 5

SHIFTS = [(0, 0)]
for i in range(1, R + 1):
    SHIFTS.extend([(-i, 0), (i, 0), (0, -i), (0, i)])

# input load row-slices: tgt rows [0,16,32,44,56), src rows [0,12,24,36,48)
TGT_CUTS = [0, 16, 32, 44, 56]
SRC_CUTS = [0, 12, 24, 36, 48]
# horizontal bank b covers h in [b*48/5, ...]; bank ready after these pieces:
#   bank0: h<=9  -> tgt rows <=13 (piece 1), src rows <=9  (piece 1)
#   bank1: h<=19 -> tgt <=23 (piece 2), src <=19 (piece 2)
#   bank2: h<=28 -> tgt <=32 (piece 3), src <=28 (piece 3)
#   bank3: h<=38 -> tgt <=42 (piece 3), src <=38 (piece 4)
#   bank4: h<=47 -> all


def build_nc():
    cdt = getattr(mybir.dt, COMPUTE_DT)
    bdt = mybir.dt.float16 if COMPUTE_DT != "float32" else F32
    nc = bacc.Bacc("TRN2", target_bir_lowering=False)
    src = nc.dram_tensor("src", [C, HSH * W], cdt, kind="ExternalInput")
    tgt = nc.dram_tensor("tgt", [C, HT * WP], cdt, kind="ExternalInput")
    # band layouts are DMA-run-friendly: vband [g, h, bank, j, h'] so one
    # DMA per (bank-group, g) writes multi-KB contiguous runs per partition
    vband = nc.dram_tensor("vband", [2, HSH, NVBANK, VSLOT, HT], bdt,
                           kind="ExternalOutput")
    hband = nc.dram_tensor("hband", [MH, len(HBASES), NHBANK * HSLOT, NH], bdt,
                           kind="ExternalOutput")

    with ExitStack() as ctx:
        tc = ctx.enter_context(tile.TileContext(nc))
        ins = ctx.enter_context(tc.tile_pool(name="ins", bufs=1))
        psum = ctx.enter_context(tc.tile_pool(name="psum", bufs=4, space="PSUM"))
        stage = ctx.enter_context(tc.tile_pool(name="stage", bufs=6))

        src_sb = ins.tile([C, HSH * W], cdt)
        tgt_sb = ins.tile([C, HT * WP], cdt)

        def load_piece(i):
            t0, t1 = TGT_CUTS[i] * WP, TGT_CUTS[i + 1] * WP
            s0, s1 = SRC_CUTS[i] * W, SRC_CUTS[i + 1] * W
            nc.sync.dma_start(out=tgt_sb[:, t0:t1], in_=tgt[:][:, t0:t1])
            nc.sync.dma_start(out=src_sb[:, s0:s1], in_=src[:][:, s0:s1])

        src3 = src_sb.rearrange("c (h w) -> c h w", w=W)
        tgt3 = tgt_sb.rearrange("c (h w) -> c h w", w=WP)

        copy_flip = [0]

        def stage_copy(dst, src_ap):
            # alternate PSUM->SBUF copies between DVE and ACT
            if copy_flip[0] % 2 == 0:
                nc.vector.tensor_copy(out=dst, in_=src_ap)
            else:
                nc.scalar.copy(out=dst, in_=src_ap)
            copy_flip[0] += 1

        def horiz_bank(bank, st, k):
            q0 = bank * HPERBANK
            pt = psum.tile([128, HSLOT * NH], F32, tag="hp")
            for g, base in enumerate(HBASES):
                for j in range(HSLOT):
                    q = q0 + g * HSLOT + j
                    h, i = divmod(q, NCH)
                    w0 = i * MH
                    nc.tensor.matmul(
                        out=pt[base:base + MH, j * NH:(j + 1) * NH],
                        lhsT=src3[:, h, w0:w0 + MH],
                        rhs=tgt3[:, h + R, w0:w0 + NH],
                        start=True, stop=True,
                        tile_position=(0, base),
                    )
            seg = HSLOT * NH
            stage_copy(st[:, k * seg:(k + 1) * seg], pt)

        def horiz_flush(st, grp):
            nb, b0 = len(grp), grp[0]
            seg = HSLOT * NH
            for g, base in enumerate(HBASES):
                nc.scalar.dma_start(
                    out=hband[:][:, g, b0 * HSLOT:(b0 + nb) * HSLOT, :],
                    in_=st[base:base + MH, :nb * seg],
                )

        def v_ng(bank, g):
            return min(VSLOT, max(0, min(VPERBANK, W - bank * VPERBANK) - g * VSLOT))

        def vert_bank(bank, st, k):
            w0 = bank * VPERBANK
            pt = psum.tile([112, VSLOT * HT], F32, tag="vp")
            for g, base in enumerate(VBASES):
                for j in range(v_ng(bank, g)):
                    w = w0 + g * VSLOT + j
                    nc.tensor.matmul(
                        out=pt[base:base + HSH, j * HT:(j + 1) * HT],
                        lhsT=src3[:, :, w],
                        rhs=tgt3[:, 0:HT, w + R],
                        start=True, stop=True,
                        tile_position=(0, base),
                    )
            seg = VSLOT * HT
            for g, base in enumerate(VBASES):
                ng = v_ng(bank, g)
                if ng > 0:
                    stage_copy(
                        st[base:base + HSH, k * seg:k * seg + ng * HT],
                        pt[base:base + HSH, :ng * HT],
                    )

        def vert_flush(st, grp):
            seg = VSLOT * HT
            for g, base in enumerate(VBASES):
                i = 0
                while i < len(grp):
                    ng = v_ng(grp[i], g)
                    if ng == 0:
                        i += 1
                        continue
                    if ng == VSLOT:
                        j = i
                        while j + 1 < len(grp) and v_ng(grp[j + 1], g) == VSLOT:
                            j += 1
                        nb = j - i + 1
                        nc.scalar.dma_start(
                            out=vband[:][g, :, grp[i]:grp[i] + nb, :, :],
                            in_=st[base:base + HSH, i * seg:(i + nb) * seg],
                        )
                        i = j + 1
                    else:
                        nc.scalar.dma_start(
                            out=vband[:][g, :, grp[i], :ng, :],
                            in_=st[base:base + HSH, i * seg:i * seg + ng * HT],
                        )
                        i += 1

        HGRP = [[0, 1, 2], [3, 4]]
        VGRP = [[0, 1, 2], [3, 4, 5], [6, 7, 8]]
        hseg, vseg = HSLOT * NH, VSLOT * HT

        # pipeline: issue loads piecewise; horizontal banks unlock as the
        # rows they need land; vertical banks need all pieces.
        load_piece(0)
        load_piece(1)
        hst = stage.tile([128, 3 * hseg], bdt, tag="hs")
        horiz_bank(0, hst, 0)
        load_piece(2)
        horiz_bank(1, hst, 1)
        load_piece(3)
        horiz_bank(2, hst, 2)
        horiz_flush(hst, HGRP[0])
        hst2 = stage.tile([128, 3 * hseg], bdt, tag="hs")
        horiz_bank(3, hst2, 0)
        horiz_bank(4, hst2, 1)
        horiz_flush(hst2, HGRP[1])
        for grp in VGRP:
            vst = stage.tile([112, 3 * vseg], bdt, tag="vs")
            for k, bank in enumerate(grp):
                vert_bank(bank, vst, k)
            vert_flush(vst, grp)

    nc.compile()
    return nc


_NC_CACHE = []


def _get_nc():
    if not _NC_CACHE:
        _NC_CACHE.append(build_nc())
    return _NC_CACHE[0]


def shard_inputs(src, tgt):
    if COMPUTE_DT == "float32":
        np_cdt = np.float32
    elif COMPUTE_DT == "float16":
        np_cdt = np.float16
    else:
        import ml_dtypes
        np_cdt = np.dtype(ml_dtypes.bfloat16)
    src = np.asarray(src, dtype=np.float32)
    tgt = np.asarray(tgt, dtype=np.float32)
    tp = np.pad(tgt, ((0, 0), (0, 0), (R, R), (R, R)))
    in_maps = []
    for core in range(NCORES):
        b, hh = divmod(core, 2)
        h0 = hh * HSH
        s = np.ascontiguousarray(src[b, :, h0:h0 + HSH, :]).reshape(C, HSH * W)
        t = np.ascontiguousarray(tp[b, :, h0:h0 + HT, :]).reshape(C, HT * WP)
        in_maps.append({"src": s.astype(np_cdt), "tgt": t.astype(np_cdt)})
    return in_maps


def extract_output(results):
    """results: list of 8 dicts with
    'vband' [2, 48, NVBANK, 9, 56], 'hband' [32, 4, NHBANK*12, 40]."""
    out = np.zeros((B, len(SHIFTS), H, W), np.float32)
    hidx = np.arange(HSH)
    midx = np.arange(MH)
    widx = np.arange(W)
    iidx = np.arange(NCH)
    for core in range(NCORES):
        b, hh = divmod(core, 2)
        h0 = hh * HSH
        # [g,h,bank,j,h'] -> [h, w=bank*18+g*9+j, h']
        vb = np.asarray(results[core]["vband"]).astype(np.float32)
        vb = vb.transpose(1, 2, 0, 3, 4).reshape(HSH, 2 * NVBANK * VSLOT, HT)
        vb = vb[:, :W, :]
        # [m,g,bank*12+j,n] -> [m, q=bank*48+g*12+j, n] -> [m,h,i,n]
        hb = np.asarray(results[core]["hband"]).astype(np.float32)
        hb = hb.reshape(MH, len(HBASES), NHBANK, HSLOT, NH)
        hb = hb.transpose(0, 2, 1, 3, 4).reshape(MH, NQ, NH)
        hb = hb.reshape(MH, HSH, NCH, NH)
        for s, (dh, dw) in enumerate(SHIFTS):
            if dw == 0:
                out[b, s, h0:h0 + HSH, :] = vb[
                    hidx[:, None], widx[None, :], (hidx + dh + R)[:, None]
                ]
            else:
                v = hb[
                    midx[:, None, None],
                    hidx[None, :, None],
                    iidx[None, None, :],
                    (midx + dw + R)[:, None, None],
                ]  # [m, h, i]
                out[b, s, h0:h0 + HSH, :] = v.transpose(1, 2, 0).reshape(HSH, W)
    return out


def kernel(src, tgt, **run_kwargs):
    nc = _get_nc()
    in_maps = shard_inputs(src, tgt)
    res = bass_utils.run_bass_kernel_spmd(
        nc, in_maps, core_ids=list(range(NCORES)), **run_kwargs
    )
    out = extract_output(res.results)
    kernel.last_result = res
    return out
